# revision 48
# baseline (speedup 1.0000x reference)
"""Multi-head attention (B=2, S=2048, D=1024, H=16, d_k=64) on 8 trn2 cores.

Sharding: batch (2) x head-groups (4 groups of 4 heads). Each core computes
its batch's full sequence for its 4 heads plus the partial output projection
(w_o row-sharded); host sums the 4 bf16 partials per batch and adds b_o.

Numerics: single-pass bf16 matmuls with fp32 PSUM accumulation everywhere
(the 2e-2 rel-err budget has ~10x headroom over bf16-grade ~2e-3 error).
exp() writes bf16 directly so the attention probabilities feed the AV
matmul with no intermediate cast.

Schedule: the exp stream on the scalar engine is the pacer (~1.11us per
[128,1024] tile, 128 tiles). Projections are issued per head-pair so the
first attention block starts as early as possible; the remaining
projections are issued in the inter-block gaps where the scalar engine
still has exp backlog. Input DMA is split across both HWDGE rings
(sync: x_k/x_q, scalar: weights + x_v + w_o).

Layout: all inputs host-pre-transposed to [d_model, seq]:
  qT/kT  = w_c @ x.T  -> [128(d' pair), 2, 2048] bf16
  vh     = x @ w_c.T  -> [128(t), tc, h, 65] bf16 (+ ones column)
  scoresT[t, s] (K=64, head pairs packed via tile_position row groups)
  atT    = exp(scoresT/8) bf16  (no max subtraction: scores ~ N(0,1))
  av     = [vh_h | 1].T @ atT   (fp32 psum, M=65: row 64 = denominators)
  out   += (av * bcast(1/denom)).T @ w_oT   (bf16 partial)
"""

import numpy as np

P = 128
S = 2048
DM = 1024
DH = 256          # head dims per core (4 heads x 64)
H = 4             # heads per core
DK = 64
MC = DM // P      # 8 m-chunks
TC = S // P       # 16 t-chunks
ST = 1024         # s-tile for scores/exp psum tiles
NST = S // ST     # 2
N_CORES = 8

_COMPILED = None


def _build():
    import concourse.bacc as bacc
    import concourse.mybir as mybir
    from concourse.tile import TileContext

    F32 = mybir.dt.float32
    BF16 = mybir.dt.bfloat16
    AF = mybir.ActivationFunctionType
    OP = mybir.AluOpType

    nc = bacc.Bacc(None, target_bir_lowering=False)

    xin = {}
    win = {}
    for t in ("q", "k", "v"):
        xin[t] = nc.dram_tensor(f"x{t}", [DM, S], BF16, kind="ExternalInput")
        # host-prepacked to the SBUF layout [p, mc, n] so the DMA moves
        # contiguous 4KB partition lines instead of 512B strided runs
        win[t] = nc.dram_tensor(f"w{t}", [P, MC * DH], BF16, kind="ExternalInput")
    # biases host-prepacked: bq/bk as [P, 2] (partition-major), bv tiled
    # to the broadcast layout so the DMAs are contiguous partition lines
    bq = nc.dram_tensor("bq", [P, 2], F32, kind="ExternalInput")
    bk = nc.dram_tensor("bk", [P, 2], F32, kind="ExternalInput")
    bv = nc.dram_tensor("bv", [P, DH], F32, kind="ExternalInput")
    wo = nc.dram_tensor("wo", [P, 2 * DM], BF16, kind="ExternalInput")
    out = nc.dram_tensor("out", [S, DM], BF16, kind="ExternalOutput")

    with TileContext(nc) as tc:
        with (
            tc.tile_pool(name="persist", bufs=1) as pp,
            tc.tile_pool(name="xfull", bufs=24) as xw,
            tc.tile_pool(name="trans", bufs=10) as xp,
            tc.tile_pool(name="athl", bufs=8) as hp,
            tc.tile_pool(name="dram", bufs=4, space="DRAM") as dp,
            tc.tile_pool(name="ps_sc", bufs=2, space="PSUM") as ps_sc,
            tc.tile_pool(name="ps_av", bufs=2, space="PSUM") as ps_av,
        ):
            qT = pp.tile([P, 2, S], BF16, name="qT")
            kT = pp.tile([P, 2, S], BF16, name="kT")
            vh = pp.tile([P, TC, H, DK + 1], BF16, name="vh")
            wo_sb = pp.tile([P, 2, DM], BF16, name="wo_sb")
            o2a = pp.tile([P, S], BF16, name="o2a")  # heads 0,1 normalized
            o2b = pp.tile([P, S], BF16, name="o2b")  # heads 2,3
            bq_sb = pp.tile([P, 2], F32, name="bq_sb")
            bk_sb = pp.tile([P, 2], F32, name="bk_sb")
            bv_bc = pp.tile([P, DH], F32, name="bv_bc")

            nc.vector.memset(vh[:, :, :, DK : DK + 1], 1.0)

            # ---------------- input DMA (two HWDGE rings) -------------------
            # sync ring: x_k then x_q (1 MB transfers). scalar ring: all
            # weights first (small), then x_v, then w_o. The scalar engine
            # only issues these before its first exp, so nothing competes
            # with the activation stream later.
            wt = {}
            for t in ("k", "q", "v"):
                w = pp.tile([P, MC, DH], BF16, name=f"w{t}_sb")
                nc.scalar.dma_start(
                    w[:], win[t][:].rearrange("p (c n) -> p c n", c=MC)
                )
                wt[t] = w
            nc.scalar.dma_start(bq_sb[:], bq[:])
            nc.scalar.dma_start(bk_sb[:], bk[:])
            nc.scalar.dma_start(bv_bc[:], bv[:])
            xt = {}
            for t in ("k", "q", "v"):
                eng = nc.scalar if t == "v" else nc.sync
                for mc in range(MC):
                    x = xw.tile([P, S], BF16, name="xc")
                    eng.dma_start(x[:], xin[t][mc * P : (mc + 1) * P, :])
                    xt[(t, mc)] = x
            nc.scalar.dma_start(wo_sb[:], wo[:].rearrange("p (c n) -> p c n", c=2))

            # preload the exp spline table set during phase A so the one-time
            # ~2.7us ACT_TABLE_LOAD doesn't sit inside the exp conveyor
            warm = xp.tile([1, 2], F32, name="xc")
            nc.vector.memset(warm[0:1, :], 0.0)
            nc.scalar.activation(warm[0:1, 0:1], warm[0:1, 1:2], AF.Exp)

            def xsl(t, mc):
                return xt[(t, mc)][:]

            # ---------------- projections, issued per pair ------------------
            def proj_kq(t, b_sb, dT, pair, st2s):
                tiles = {}
                for st2 in st2s:
                    pool = (ps_sc, ps_av)[st2]
                    tiles[st2] = pool.tile([P, ST], F32, name=("sc", "av")[st2])
                for mc in range(MC):
                    for hf in range(2):
                        for st2 in st2s:
                            nc.tensor.matmul(
                                tiles[st2][:, hf * 512 : (hf + 1) * 512],
                                wt[t][:, mc, pair * P : (pair + 1) * P],
                                xsl(t, mc)[
                                    :, st2 * ST + hf * 512 : st2 * ST + (hf + 1) * 512
                                ],
                                start=(mc == 0),
                                stop=(mc == MC - 1),
                            )
                for st2 in st2s:
                    sl = (slice(None), pair, slice(st2 * ST, (st2 + 1) * ST))
                    nc.vector.tensor_scalar(
                        out=dT[sl], in0=tiles[st2][:], scalar1=b_sb[:, pair : pair + 1],
                        scalar2=None, op0=OP.add,
                    )

            def proj_vh():
                for tcc in range(TC):
                    pool = (ps_sc, ps_av)[tcc % 2]
                    ps = pool.tile([P, DH], F32, name=("sc", "av")[tcc % 2])
                    for mc in range(MC):
                        nc.tensor.matmul(
                            ps[:],
                            xsl("v", mc)[:, tcc * P : (tcc + 1) * P],
                            wt["v"][:, mc, :],
                            start=(mc == 0),
                            stop=(mc == MC - 1),
                        )
                    nc.vector.tensor_tensor(
                        out=vh[:, tcc, :, 0:DK],
                        in0=ps[:].rearrange("p (h d) -> p h d", h=H),
                        in1=bv_bc[:].rearrange("p (h d) -> p h d", h=H),
                        op=OP.add,
                    )

            # ---------------- attention block -------------------------------
            pending = []  # deferred thunks drained one per tcc step

            def drain_one():
                if pending:
                    pending.pop(0)()

            def attn_block(pair, st2, last=False):
                o2h = (o2a, o2b)[pair]
                s0 = st2 * ST
                avs = [ps_av.tile([P, ST], F32, name="av") for _ in range(2)]
                ats = {}

                def scores_exp(tcc):
                    for hi2 in range(2):
                        rows = slice(DK * hi2, DK * (hi2 + 1))
                        sc = ps_sc.tile([P, ST], F32, name="sc")
                        for hf in range(2):
                            nc.tensor.matmul(
                                sc[:, hf * 512 : (hf + 1) * 512],
                                kT[rows, pair, tcc * P : (tcc + 1) * P],
                                qT[rows, pair, s0 + hf * 512 : s0 + (hf + 1) * 512],
                                start=True,
                                stop=True,
                                tile_position=(DK * hi2, 0),
                            )
                        at = hp.tile([P, ST], BF16, name="at")
                        nc.scalar.activation(at[:], sc[:], AF.Exp)
                        ats[(tcc, hi2)] = at

                def av_mm(tcc):
                    for hi2 in range(2):
                        at = ats.pop((tcc, hi2))
                        h = 2 * pair + hi2
                        for hf in range(2):
                            nc.tensor.matmul(
                                avs[hi2][0 : DK + 1, hf * 512 : (hf + 1) * 512],
                                vh[:, tcc, h, :],
                                at[:, hf * 512 : (hf + 1) * 512],
                                start=(tcc == 0),
                                stop=(tcc == TC - 1),
                            )

                scores_exp(0)
                scores_exp(1)
                for tcc in range(2, TC):
                    scores_exp(tcc)
                    av_mm(tcc - 2)
                    drain_one()
                av_mm(TC - 2)
                av_mm(TC - 1)
                for n2 in pending[:]:
                    n2()
                pending.clear()

                # normalize part 1: copy unnormalized rows out of PSUM and
                # compute the reciprocal of the denominator row (the
                # reciprocal needs a 1-partition SBUF source; wider or
                # PSUM-sourced forms fail walrus codegen).
                # in the last block the scalar engine is idle (conveyor
                # done), so its copies shorten the normalize critical path;
                # in earlier blocks they would stall the exp stream.
                norm_rs = []
                for hi2 in range(2):
                    av = avs[hi2]
                    rows = slice(DK * hi2, DK * (hi2 + 1))
                    u = xp.tile([P, ST], F32, name="xc")
                    dsb = xp.tile([1, ST], F32, name="xc")
                    if last:
                        nc.scalar.copy(u[rows, :], av[0:DK, :])
                        nc.scalar.copy(dsb[0:1, :], av[DK : DK + 1, :])
                    else:
                        nc.vector.tensor_copy(u[rows, :], av[0:DK, :])
                        nc.vector.tensor_copy(dsb[0:1, :], av[DK : DK + 1, :])
                    rsb = xp.tile([1, ST], F32, name="xc")
                    scr = xp.tile([1, ST], F32, name="xc")
                    nc.vector.reciprocal_approx_accurate(
                        rsb[0:1, :], dsb[0:1, :], scr[0:1, :]
                    )
                    rdr = dp.tile([1, ST], F32, name="rdr")
                    nc.sync.dma_start(rdr[0:1, :], rsb[0:1, :])
                    norm_rs.append((rdr, u))

                # normalize part 2 (deferred): broadcast the reciprocal via
                # DRAM round-trip, then fused multiply+cast to bf16.
                for hi2 in range(2):
                    rdr, u = norm_rs[hi2]
                    rows = slice(DK * hi2, DK * (hi2 + 1))
                    sl = (rows, slice(s0, s0 + ST))
                    rb = xp.tile([P, ST], F32, name="xc")

                    def t1(rdr=rdr, rb=rb, rows=rows):
                        nc.sync.dma_start(
                            rb[rows, :], rdr[0:1, :].to_broadcast((DK, ST))
                        )

                    def t2(u=u, rb=rb, rows=rows, sl=sl, o2h=o2h):
                        nc.vector.tensor_tensor(
                            out=o2h[sl], in0=u[rows, :], in1=rb[rows, :],
                            op=OP.mult,
                        )

                    pending.extend([t1, t2])

            # ---------------- issue order -----------------------------------
            # monolithic phase A: the attention blocks are exp-paced with
            # ~100% scalar-engine duty, so there is no backlog to hide
            # projections under — splitting them into the stream only adds
            # exposed gaps (and PSUM capacity forbids true interleaving).
            proj_kq("k", bk_sb, kT, 0, (0, 1))
            proj_kq("k", bk_sb, kT, 1, (0, 1))
            proj_kq("q", bq_sb, qT, 0, (0, 1))
            proj_kq("q", bq_sb, qT, 1, (0, 1))
            proj_vh()
            attn_block(0, 0)
            attn_block(0, 1)
            attn_block(1, 0)
            attn_block(1, 1, last=True)

            # ---------------- output projection -----------------------------
            # st7 0..7 read s-columns < 1024, whose normalize (st2=0 blocks)
            # has already drained; emit them before the final norm2 drain so
            # the PE covers the last block's normalize latency.
            st7_order = list(range(TC // 2)) + [-1] + list(range(TC // 2, TC))
            for st7 in st7_order:
                if st7 == -1:
                    for n2 in pending[:]:
                        n2()
                    pending.clear()
                    continue
                if st7 < TC // 2:
                    drain_one()
                # ps_sc is free once the last block's scores drain, so wo
                # overlaps the final normalize instead of waiting on ps_av;
                # the sc,sc,av,av pattern keeps a 4-deep rotation while the
                # first TWO chunks avoid ps_av, which is still held by the
                # last block's accumulators until its normalize copies
                # finish (~2us PE gap otherwise, seen in the trace).
                nm, pool = (
                    ("sc", ps_sc), ("sc", ps_sc), ("av", ps_av), ("av", ps_av)
                )[st7 % 4]
                of_ps = pool.tile([P, ST], F32, name=nm)
                for c in range(2):
                    o2h = (o2a, o2b)[c]
                    for nh in range(2):
                        nc.tensor.matmul(
                            of_ps[:, nh * 512 : (nh + 1) * 512],
                            o2h[:, st7 * P : (st7 + 1) * P],
                            wo_sb[:, c, nh * 512 : (nh + 1) * 512],
                            start=(c == 0),
                            stop=(c == 1),
                        )
                of = xp.tile([P, ST], BF16, name="xc")
                # split the psum->bf16 casts between DVE and the (post-
                # conveyor idle) scalar engine: the cast chain, not the PE,
                # paces this phase.
                if st7 % 2:
                    nc.scalar.copy(of[:], of_ps[:])
                else:
                    nc.vector.tensor_copy(of[:], of_ps[:])
                eng = nc.scalar if st7 % 2 else nc.sync
                eng.dma_start(out[st7 * P : (st7 + 1) * P, :], of[:])

    nc.compile()
    return nc


def _get_nc():
    global _COMPILED
    if _COMPILED is None:
        _COMPILED = _build()
    return _COMPILED


def _bf16(x):
    import ml_dtypes

    return np.ascontiguousarray(x.astype(ml_dtypes.bfloat16))


def _make_in_maps(q, k, v, w_q, b_q, w_k, b_k, w_v, b_v, w_o, b_o):
    q = np.asarray(q, np.float32)
    k = np.asarray(k, np.float32)
    v = np.asarray(v, np.float32)
    xs = {}
    for t, arr in (("q", q), ("k", k), ("v", v)):
        for b in range(2):
            xs[(t, b)] = _bf16(np.ascontiguousarray(arr[b].T))
    # fold the 1/sqrt(d_k) score scale into the q projection so the exp
    # activation runs with scale=1
    ws = {"q": np.asarray(w_q, np.float32) * 0.125,
          "k": np.asarray(w_k, np.float32),
          "v": np.asarray(w_v, np.float32)}
    bs = {"q": np.asarray(b_q, np.float32) * 0.125,
          "k": np.asarray(b_k, np.float32),
          "v": np.asarray(b_v, np.float32)}
    w_o = np.asarray(w_o, np.float32)
    in_maps = []
    for core in range(N_CORES):
        b, hg = divmod(core, 4)
        sl = slice(hg * DH, (hg + 1) * DH)
        m = {}
        for t in ("q", "k", "v"):
            m[f"x{t}"] = xs[(t, b)]
            # pack w.T [DM, DH] as [p, mc*DH]: row p holds chunks mc.
            wT = ws[t][sl, :].T.reshape(MC, P, DH).transpose(1, 0, 2)
            m[f"w{t}"] = _bf16(wT.reshape(P, MC * DH))
            bsl = bs[t][sl]
            if t == "v":
                m[f"b{t}"] = np.ascontiguousarray(
                    np.tile(bsl[None, :], (P, 1)).astype(np.float32)
                )
            else:
                m[f"b{t}"] = np.ascontiguousarray(
                    bsl.reshape(2, P).T.astype(np.float32)
                )
        woT = w_o[:, sl].T.reshape(2, P, DM).transpose(1, 0, 2)
        m["wo"] = _bf16(woT.reshape(P, 2 * DM))
        in_maps.append(m)
    return in_maps


def run(inputs, trace=False):
    from concourse.bass_utils import run_bass_kernel_spmd

    nc = _get_nc()
    in_maps = _make_in_maps(**inputs)
    res = run_bass_kernel_spmd(
        nc, in_maps, core_ids=list(range(N_CORES)), trace=trace
    )
    b_o = np.asarray(inputs["b_o"], np.float32)
    full = np.empty((2, S, DM), np.float32)
    for b in range(2):
        acc = res.results[4 * b]["out"].astype(np.float32)
        for hg in range(1, 4):
            acc = acc + res.results[4 * b + hg]["out"].astype(np.float32)
        full[b] = acc + b_o[None, :]
    return full, res


def kernel(**inputs) -> np.ndarray:
    full, _ = run(inputs, trace=False)
    return full
